# revision 69
# baseline (speedup 1.0000x reference)
"""Trainium2 Bass kernel for nn_AggregationGPE.

Math: the reference's curvature-softmax weights are identical along the
softmax axis, so softmax yields exactly 0.25 per basis and the output is
    out[p, d*128+j] = 0.25*(exp(-50*(x-g_j)^2) + cos(x-t_j) + sin(x-t_j)
                            + tanh(x-h_j)),   x = xyz[p, d]
with g_j = inner linspace(-1,1), t_j = pi*g_j, h_j = 2*g_j.
neighbor_xyz does not influence the output and is never touched.

Device strategy (8 cores, pure data parallel over the 65536 points):
  - gaussian: Square + Exp on ACT (0.25 folded into the exp bias), tanh on
    ACT; all three live in the single 'exp_and_others' table set -> no
    table-set switching in steady state.
  - cos+sin is rank-2: cos(x-t)+sin(x-t) = sinx*(sin t+cos t)+cosx*(cos t-sin t).
    Per-row sin(x)/cos(x) (explicitly range-reduced; HW Sin is only valid on
    ~[-3.3, 3.3]) are transposed via PE and contracted with a block-diagonal
    constant by K=6 matmuls into PSUM on the otherwise idle TensorE.
  - broadcast-subtracts (vals - x) split across DVE and GPSIMD with one
    owning engine per tile (mixed writers create spurious Tile ordering);
    fused combine via scalar_tensor_tensor + tensor_tensor on DVE.
  - tapered batch sizes shorten pipeline ramp and drain.
"""

import math
import time

import numpy as np

import bass_rust
import concourse.bass as bass
import concourse.mybir as mybir
from concourse.tile import TileContext
from concourse.bass_utils import run_bass_kernel_spmd

AF = mybir.ActivationFunctionType
OP = mybir.AluOpType
FP = mybir.dt.float32

N_CORES = 8
PTS = 65536            # 16*4096 points
PPC = PTS // N_CORES   # 8192 points per core
NT = PPC // 128        # 64 point-tiles per core
NBLK = (NT + 2) // 3   # 22 transpose blocks (3 tiles per 128-row block)
NTP = NBLK * 3         # 66 tile slots (2 padding tiles)

NF = 128
TWO_PI = 2.0 * math.pi
INV_2PI = 1.0 / TWO_PI
LN4 = math.log(4.0)

_ctr = [0]


def _split_waits(nc, maxw=1):
    """This walrus build accepts a single sync-wait per instruction; hoist
    extras emitted by the Tile scheduler onto NOPs placed just before."""
    for f in nc.m.functions:
        for bb in f.blocks:
            if not any(
                i.sync_info is not None and len(i.sync_info.on_wait) > maxw
                for i in bb.instructions
            ):
                continue
            new = []
            for inst in bb.instructions:
                si = inst.sync_info
                if si is not None and len(si.on_wait) > maxw:
                    waits = list(si.on_wait)
                    keep = waits[-maxw:]
                    hoist = waits[:-maxw]
                    for j in range(0, len(hoist), maxw):
                        _ctr[0] += 1
                        nop = mybir.InstNoOp(name=f"WSPLIT-{_ctr[0]}", ins=[], outs=[])
                        nop.engine = inst.engine
                        nop.sync_info = bass_rust.SyncInfo(
                            on_wait=hoist[j : j + maxw], on_update=[]
                        )
                        nc.register_instruction(nop, overwrite=True)
                        new.append(nop)
                    si.on_wait.clear()
                    for w in keep:
                        si.on_wait.append(w)
                new.append(inst)
            bb.instructions = new


def _host_consts():
    j = np.arange(NF, dtype=np.float64)
    g = (2.0 / (NF + 1)) * (j + 1.0) - 1.0     # inner linspace(-1, 1, NF+2)
    t = math.pi * g
    h = 2.0 * g
    cvals = np.empty((128, 2 * NF), dtype=np.float32)
    cvals[:, 0:NF] = g.astype(np.float32)
    cvals[:, NF:] = h.astype(np.float32)
    A = 0.25 * (np.sin(t) + np.cos(t))
    B = 0.25 * (np.cos(t) - np.sin(t))
    crhs = np.zeros((128, 384), dtype=np.float32)
    for q0 in (0, 32, 64):
        for d in range(3):
            crhs[q0 + 2 * d, d * NF : (d + 1) * NF] = A.astype(np.float32)
            crhs[q0 + 2 * d + 1, d * NF : (d + 1) * NF] = B.astype(np.float32)
    ident = np.eye(128, dtype=np.float32)
    return cvals, crhs, ident


def _build():
    nc = bass.Bass()
    xs = nc.dram_tensor("xs", [PPC, 3], FP, kind="ExternalInput")
    cvals_d = nc.dram_tensor("cvals", [128, 256], FP, kind="ExternalInput")
    crhs_d = nc.dram_tensor("crhs", [128, 384], FP, kind="ExternalInput")
    ident_d = nc.dram_tensor("ident", [128, 128], FP, kind="ExternalInput")
    out_d = nc.dram_tensor("out", [PPC, 384], FP, kind="ExternalOutput")

    out_v = out_d[:, :].rearrange("(t p) c -> p t c", p=128)  # [128, NT, 384]

    with TileContext(nc) as tc:
        with tc.tile_pool(name="const", bufs=1) as cpool, tc.tile_pool(
            name="setup", bufs=1
        ) as spool:
            cvals = cpool.tile([128, 256], FP)
            crhs = cpool.tile([128, 384], FP)
            ident = cpool.tile([128, 128], FP)
            xq = cpool.tile([128, 3 * NTP], FP)        # [p, (t, d)], 198 cols
            sincosT = cpool.tile([128, NBLK * 128], FP)
            b_halfpi = cpool.tile([128, 1], FP)
            b_mln4 = cpool.tile([128, 1], FP)
            b_zero = cpool.tile([128, 1], FP)
            nc.vector.memset(b_halfpi[:, :], math.pi / 2)
            nc.vector.memset(b_mln4[:, :], -LN4)
            nc.vector.memset(b_zero[:, :], 0.0)
            nc.gpsimd.memset(xq[:, 3 * NT :], 0.0)
            xs_v = xs[:, :].rearrange("(t p) d -> p t d", p=128)
            xq_v = xq[:, 0 : 3 * NT].rearrange("p (t d) -> p t d", d=3)
            # first xq slice + cvals lead so batch 0's subs unblock earliest;
            # ident/crhs (needed only by transposes/matmuls) load last
            nc.sync.dma_start(xq_v[:, 0:4, :], xs_v[:, 0:4, :])
            nc.sync.dma_start(cvals[:, :], cvals_d[:, :])
            xq_cuts = [4, 16, 32, 48, NT]
            for q in range(len(xq_cuts) - 1):
                a, bnd = xq_cuts[q], xq_cuts[q + 1]
                nc.sync.dma_start(xq_v[:, a:bnd, :], xs_v[:, a:bnd, :])
            nc.sync.dma_start(ident[:, :], ident_d[:, :])
            nc.sync.dma_start(crhs[:, :], crhs_d[:, :])

            # ---- setup: per-row sin(x), cos(x), range-reduced ----
            # (spool stays open for the whole kernel: closing it would let
            # the steady-state tiles reuse these addresses and serialize the
            # first batches behind the PE transposes)
            with tc.tile_pool(name="ptp", bufs=2, space="PSUM") as tppool:
                k1 = spool.tile([128, 3 * NTP], mybir.dt.int32, tag="k1")
                k2 = spool.tile([128, 3 * NTP], mybir.dt.int32, tag="k2")
                arg1 = spool.tile([128, 3 * NTP], FP, tag="a1")
                arg2 = spool.tile([128, 3 * NTP], FP, tag="a2")
                # sincos_pre[p, blk*128 + tb*32 + 2d + s] = sin/cos of x[t=3blk+tb, d]
                pre = spool.tile([128, NBLK * 128], FP, tag="pre")
                nc.gpsimd.memset(pre[:, :], 0.0)
                pre_v = (
                    pre[:, :]
                    .rearrange("p (b x) -> p b x", x=128)[:, :, 0:96]
                    .rearrange("p b (tb r) -> p b tb r", r=32)[:, :, :, 0:6]
                    .rearrange("p b tb (d s) -> p b tb d s", s=2)
                )
                a1_v = arg1[:, :].rearrange("p (b tb d) -> p b tb d", tb=3, d=3)
                a2_v = arg2[:, :].rearrange("p (b tb d) -> p b tb d", tb=3, d=3)
                # run the range-reduction + Sin chain in halves so the first
                # transposes start before the second xq half has landed
                HB2 = NBLK // 2
                for hh in range(2):
                    cs = slice(hh * HB2 * 9, (3 * NTP) if hh else HB2 * 9)
                    bs = slice(hh * HB2, NBLK if hh else HB2)
                    nc.vector.tensor_scalar(
                        k1[:, cs], xq[:, cs], INV_2PI, None, OP.mult
                    )
                    nc.vector.scalar_tensor_tensor(
                        arg1[:, cs], k1[:, cs], -TWO_PI, xq[:, cs], OP.mult, OP.add
                    )
                    nc.vector.tensor_scalar(
                        k2[:, cs], xq[:, cs], INV_2PI, 0.25, OP.mult, OP.add
                    )
                    nc.vector.scalar_tensor_tensor(
                        arg2[:, cs], k2[:, cs], -TWO_PI, xq[:, cs], OP.mult, OP.add
                    )
                    nc.scalar.activation(
                        pre_v[:, bs, :, :, 0], a1_v[:, bs], AF.Sin, bias=b_zero[:, :]
                    )
                    nc.scalar.activation(
                        pre_v[:, bs, :, :, 1], a2_v[:, bs], AF.Sin,
                        bias=b_halfpi[:, :],
                    )

                TPG = 4  # transposes per PSUM bank -> one wide DVE copy each
                for b0 in range(0, NBLK, TPG):
                    n = min(TPG, NBLK - b0)
                    ptp = tppool.tile([128, TPG * 128], FP, tag="ptp")
                    for i in range(n):
                        b = b0 + i
                        nc.tensor.transpose(
                            ptp[:, i * 128 : (i + 1) * 128],
                            pre[:, b * 128 : (b + 1) * 128],
                            ident[:, :],
                        )
                    nc.vector.tensor_copy(
                        sincosT[:, b0 * 128 : (b0 + n) * 128], ptp[:, 0 : n * 128]
                    )

            # ---- steady state ----
            with tc.tile_pool(name="work", bufs=2) as wpool, tc.tile_pool(
                name="sq1", bufs=1
            ) as sqpool, tc.tile_pool(name="pmm", bufs=2, space="PSUM") as mmpool:
                # tapered batch sizes: small batches at the start shorten the
                # pipeline ramp, small ones at the end shorten the drain
                sizes = [2, 6] + [8] * ((NT - 16) // 8) + [6, 2]
                assert sum(sizes) == NT
                t0 = 0
                for b, T in enumerate(sizes):
                    nk = 3 * T
                    if T <= 2:
                        nd = nk          # tiny batches: all-DVE subs
                    elif b == 1:
                        nd = nk // 2     # ramp batch: extra DVE share so ACT
                                         # isn't gated on Pool's sub stream
                    else:
                        nd = max(1, (nk * 6) // 24)
                    tsub_d = wpool.tile([128, nd * 256], FP, tag="tsub_d")
                    if nk > nd:
                        tsub_p = wpool.tile([128, (nk - nd) * 256], FP, tag="tsub_p")
                    else:
                        tsub_p = None
                    for kd in range(nk):
                        t = t0 + kd // 3
                        d = kd % 3
                        if kd < nd:
                            eng, tile, o = nc.vector, tsub_d, kd
                        else:
                            eng, tile, o = nc.gpsimd, tsub_p, kd - nd
                        eng.tensor_scalar(
                            tile[:, o * 256 : (o + 1) * 256],
                            cvals[:, :],
                            xq[:, 3 * t + d : 3 * t + d + 1],
                            None,
                            OP.subtract,
                        )
                    td_v = tsub_d[:, :].rearrange("p (kd w) -> p kd w", w=256)
                    sq = sqpool.tile([128, nk * 128], FP, tag="sq")
                    sq_v = sq[:, :].rearrange("p (kd j) -> p kd j", j=128)
                    nc.scalar.activation(
                        sq_v[:, 0:nd, :], td_v[:, :, 0:128], AF.Square,
                        bias=b_zero[:, :],
                    )
                    expo = wpool.tile([128, nk * 128], FP, tag="expo")
                    tanho = wpool.tile([128, nk * 128], FP, tag="tanho")
                    tanho_v = tanho[:, :].rearrange("p (kd j) -> p kd j", j=128)
                    nc.scalar.activation(
                        tanho_v[:, 0:nd, :], td_v[:, :, 128:256], AF.Tanh,
                        bias=b_zero[:, :], scale=-1.0,
                    )
                    if tsub_p is not None:
                        tp_v = tsub_p[:, :].rearrange("p (kd w) -> p kd w", w=256)
                        nc.scalar.activation(
                            sq_v[:, nd:, :], tp_v[:, :, 0:128], AF.Square,
                            bias=b_zero[:, :],
                        )
                        nc.scalar.activation(
                            tanho_v[:, nd:, :], tp_v[:, :, 128:256], AF.Tanh,
                            bias=b_zero[:, :], scale=-1.0,
                        )
                    nc.scalar.activation(
                        expo[:, :], sq[:, :], AF.Exp, bias=b_mln4[:, :], scale=-50.0
                    )

                    q1 = wpool.tile([128, nk * 128], FP, tag="q1")
                    HB = T // 2
                    for h in range(2):
                        pmm = mmpool.tile([128, HB * 512], FP, tag="pmm")
                        for i in range(HB):
                            t = t0 + h * HB + i
                            q0 = 32 * (t % 3)
                            bcol = t // 3
                            nc.tensor.matmul(
                                pmm[:, i * 512 : i * 512 + 384],
                                sincosT[q0 : q0 + 6, bcol * 128 : (bcol + 1) * 128],
                                crhs[q0 : q0 + 6, :],
                            )
                        pmm_v = pmm[:, :].rearrange("p (i w) -> p i w", w=512)[
                            :, :, 0:384
                        ]
                        sl = slice(h * HB * 384, (h + 1) * HB * 384)
                        tanho_p = tanho[:, sl].rearrange("p (i w) -> p i w", w=384)
                        q1_vh = q1[:, sl].rearrange("p (i w) -> p i w", w=384)
                        nc.vector.scalar_tensor_tensor(
                            q1_vh, tanho_p, 0.25, pmm_v, OP.mult, OP.add
                        )
                    ob = wpool.tile([128, nk * 128], FP, tag="ob")
                    if b >= len(sizes) - 2:
                        # final batch: per-tile combine+store shortens the
                        # kernel-tail dependency chain
                        for i in range(T):
                            cs2 = slice(i * 384, (i + 1) * 384)
                            nc.vector.tensor_tensor(
                                ob[:, cs2], expo[:, cs2], q1[:, cs2], OP.add
                            )
                            nc.sync.dma_start(
                                out_v[:, t0 + i : t0 + i + 1, :],
                                ob[:, cs2].rearrange("p (t c) -> p t c", c=384),
                            )
                    else:
                        nc.vector.tensor_tensor(ob[:, :], expo[:, :], q1[:, :], OP.add)
                        nc.sync.dma_start(
                            out_v[:, t0 : t0 + T, :],
                            ob[:, :].rearrange("p (t c) -> p t c", c=384),
                        )
                    t0 += T

    _split_waits(nc)
    return nc


_CACHE = {}


def kernel(xyz: np.ndarray, neighbor_xyz: np.ndarray = None, **_) -> np.ndarray:
    if "nc" not in _CACHE:
        _CACHE["nc"] = _build()
        _CACHE["consts"] = _host_consts()
    nc = _CACHE["nc"]
    cvals, crhs, ident = _CACHE["consts"]

    xyz = np.asarray(xyz)
    B, N = xyz.shape[0], xyz.shape[1]
    assert B * N == PTS and xyz.shape[2] == 3, xyz.shape
    flat = np.ascontiguousarray(xyz.reshape(PTS, 3).astype(np.float32, copy=False))
    in_maps = []
    for c in range(N_CORES):
        in_maps.append(
            {
                "xs": np.ascontiguousarray(flat[c * PPC : (c + 1) * PPC]),
                "cvals": cvals,
                "crhs": crhs,
                "ident": ident,
            }
        )
    res = None
    last_exc = None
    for attempt in range(3):
        try:
            res = run_bass_kernel_spmd(nc, in_maps, core_ids=list(range(N_CORES)))
            break
        except Exception as e:  # transient NRT/axon device errors
            last_exc = e
            time.sleep(10 * (attempt + 1))
    if res is None:
        raise last_exc
    _CACHE["last_result"] = res
    out = np.concatenate([r["out"] for r in res.results], axis=0)
    return out.reshape(xyz.shape[0], xyz.shape[1], 384)
